# revision 33
# baseline (speedup 1.0000x reference)
"""Fused pre-norm attention layer (B=4, L=2048, D=1024, H=16, E=64) on 8 trn2 cores.

v3: DoubleRow scores + host-folded weights.
  - weights pre-scaled by LN weight on host; aug rank-2 rows (colsum,
    bias-row) uploaded fp8; aug matmuls run fp8 DoubleRow (0.5 c/r)
  - scores matmuls fp8 DoubleRow: KT/QT shuffled into [32,2,*] pair-
    interleaved layout (e = j*32+q) via cheap SBUF->SBUF DMAs
  - softmax exp split 3 ways: ACT table-exp, DVE fast-exp, Pool fast-exp
  - bf16 uploads (natural for bn_stats, transposed for matmul operands)
  - AV with fp8 DoubleRow (2 s-tiles per matmul), ones-column denominator
  - out1 (residual Q + attention) bf16; per-head-pair spill + head-mixed
    gather + final LN + Wo projection, pipelined with attention
"""

import numpy as np

import concourse.bass as bass
import concourse.mybir as mybir
import concourse.tile as tile
from concourse import bacc
from concourse.bass_utils import run_bass_kernel_spmd
from concourse.masks import make_identity

F32 = mybir.dt.float32
BF16 = mybir.dt.bfloat16
F8E4 = mybir.dt.float8e4
I8 = mybir.dt.int8
I32 = mybir.dt.int32
AF = mybir.ActivationFunctionType
ALU = mybir.AluOpType
DR = mybir.MatmulPerfMode.DoubleRow

B, L, D, H, E = 4, 2048, 1024, 16, 64
LH = L // 2
S = L
HE = H * E
EPS = 1e-5
NKT = D // 128       # 8
NLT = LH // 128      # 8
NST = S // 128       # 16
HP = H // 2          # 8 head pairs
SCALE = 1.0 / np.sqrt(E)
LOG2E = 1.4426950408889634
CEXP = 0.45
# exp engine assignment per (h01, st): ACT table-exp vs DVE fast-exp.
# (Pool/GPSIMD cannot read PSUM and PSUM cannot be DMA'd, so exp is ACT+DVE.)
EXP_ENG = {
    0: {1: "dve", 3: "dve", 5: "dve", 7: "dve", 9: "dve", 11: "dve",
        13: "dve"},
    1: {0: "dve", 2: "dve", 4: "dve", 6: "dve", 10: "dve", 12: "dve",
        14: "dve"},
}


def _rsqrt_dve(nc, sm, out, var_ap, T, tag, eng=None):
    """out = 1/sqrt(var + EPS) on DVE or Pool (Quake initial guess + 2
    Newton steps) so the ACT engine never leaves the exp table set."""
    e = eng if eng is not None else nc.vector
    ALUo = mybir.AluOpType
    veps = sm.tile([128, T], F32, tag=f"ve_{tag}", name=f"ve_{tag}")
    e.tensor_scalar(veps[:], var_ap, EPS, None, op0=ALUo.add)
    s1 = sm.tile([128, T], I32, tag=f"s1_{tag}", name=f"s1_{tag}")
    e.tensor_scalar(
        s1[:], veps[:].bitcast(I32), 1, None, op0=ALUo.logical_shift_right
    )
    e.tensor_scalar(
        s1[:], s1[:], -1, 0x5F3759DF, op0=ALUo.mult, op1=ALUo.add
    )
    r = s1[:].bitcast(F32)
    t = sm.tile([128, T], F32, tag=f"t_{tag}", name=f"t_{tag}")
    for _ in range(2):
        e.tensor_mul(t[:], veps[:], r)
        e.tensor_mul(t[:], t[:], r)
        e.tensor_scalar(t[:], t[:], -0.5, 1.5, op0=ALUo.mult, op1=ALUo.add)
        e.tensor_mul(out, r, t[:])
        r = out
    return out


def build_nc(skip=(), bias_o=False):
    nc = bacc.Bacc("TRN2", target_bir_lowering=False, debug=False, num_devices=8)

    qn_ = nc.dram_tensor("qn_", [LH, D], F8E4, kind="ExternalInput")
    kn_ = nc.dram_tensor("kn_", [S, D], F8E4, kind="ExternalInput")
    vn_ = nc.dram_tensor("vn_", [S, D], F8E4, kind="ExternalInput")
    qT = nc.dram_tensor("qT", [D, LH], BF16, kind="ExternalInput")
    kT = nc.dram_tensor("kT", [D, S], F8E4, kind="ExternalInput")
    vT = nc.dram_tensor("vT", [D, S], F8E4, kind="ExternalInput")
    # weights pre-scaled by LN weight on host
    Wq = nc.dram_tensor("Wq", [D, HE], BF16, kind="ExternalInput")
    Wk = nc.dram_tensor("Wk", [D, HE], F8E4, kind="ExternalInput")
    Wv = nc.dram_tensor("Wv", [D, HE], F8E4, kind="ExternalInput")
    Wo = nc.dram_tensor("Wo", [HE, D], BF16, kind="ExternalInput")
    # aug value rows per projection: [1, 2, n] fp8 = [colsum(w); nb@W + b]
    arq_d = nc.dram_tensor("arq_d", [1, 2, HE], F8E4, kind="ExternalInput")
    ark_d = nc.dram_tensor("ark_d", [1, 2, HE], F8E4, kind="ExternalInput")
    arv_d = nc.dram_tensor("arv_d", [1, 2, HE], F8E4, kind="ExternalInput")
    co_d = nc.dram_tensor("co_d", [1, D], F32, kind="ExternalInput")
    out1d = nc.dram_tensor("out1d", [LH, HE], BF16)
    F = nc.dram_tensor("F", [LH, D], F32, kind="ExternalOutput")

    with tile.TileContext(nc) as tc, nc.allow_low_precision(
        reason="bf16/fp8 attention; tolerance budget verified vs reference"
    ):
        with (
            tc.tile_pool(name="res", bufs=1) as res,
            tc.tile_pool(name="sm", bufs=2) as sm,
        ):
            identb = res.tile([128, 128], BF16, tag="identb", name="identb")
            make_identity(nc, identb[:])
            ones_row = res.tile([1, 128], BF16, tag="ones_row", name="ones_row")
            nc.vector.memset(ones_row[:], 1.0)

            kTt = res.tile([128, NKT, S], F8E4, tag="kTt", name="kTt")

            out1 = res.tile([128, NLT, HE], BF16, tag="out1", name="out1")
            QT = res.tile([128, HP, LH], F8E4, tag="QT", name="QT")
            QTdr = res.tile([64, HP, 2, LH], F8E4, tag="QTdr", name="QTdr")
            Vaug = res.tile([128, NST, H, E + 1], F8E4, tag="Vaug", name="Vaug")
            nc.vector.memset(Vaug[:, :, :, E : E + 1], 1.0)
            wpk = res.tile([128, NKT, HE], F8E4, tag="wpk", name="wpk")
            wpo = res.tile([128, NKT, D], BF16, tag="wpo", name="wpo")
            ar8 = {}   # name -> [1, 2, n] fp8 rows [S=colsum(wp); c=nb@W+bias]
            aug8 = {}  # name -> [1, 2, T, 128] fp8 rows [-mu; 1/rstd]
            rstd = {}  # name -> [128, T]
            if bias_o:
                co_b = res.tile([1, D], BF16, tag="co_b", name="co_b")

            with tc.tile_pool(name="wqvw", bufs=1) as wqvw:
              with (
                  tc.tile_pool(name="pp", bufs=2, space="PSUM") as pp,
                  tc.tile_pool(name="wk", bufs=1) as wk,
              ):
                # ---------- LN stats ----------
                def do_stats(name, xd, T, psum_pool, psum_tag, stp, t0=0, na=0):
                    # first T-na tiles on DVE bn_stats; last na on the ACT
                    # accumulator (Sum, SumSq) with a vectorized fixup
                    mv = res.tile(
                        [128, T, 2], F32, tag=f"mv_{name}", name=f"mv_{name}"
                    )
                    nd = T - na
                    for t in range(T):
                        xt = stp.tile(
                            [128, D], F8E4,
                            tag=f"xn{'a' if t >= nd else ''}{t % 4}", name="xnat"
                        )
                        nc.scalar.dma_start(
                            xt[:], xd[(t0 + t) * 128 : (t0 + t + 1) * 128, :]
                        )
                        if t < nd:
                            bstat = sm.tile(
                                [128, 2, 6], F32, tag="bstat", name="bstat"
                            )
                            for i in range(2):
                                nc.vector.bn_stats(
                                    bstat[:, i, :], xt[:, i * 512 : (i + 1) * 512]
                                )
                            nc.vector.bn_aggr(mv[:, t, :], bstat[:])
                        else:
                            scr = stp.tile(
                                [128, D], BF16, tag=f"scr{t % 2}", name="scr"
                            )
                            nc.scalar.activation(
                                scr[:], xt[:], AF.Copy, accum_out=mv[:, t, 0:1]
                            )
                            nc.scalar.activation(
                                scr[:], xt[:], AF.Square, accum_out=mv[:, t, 1:2]
                            )
                    if na:
                        # mu = S0/D, var = S1/D - mu^2 on the ACT-owned tail
                        mva = mv[:, nd:T, :]
                        nc.vector.tensor_scalar_mul(mva, mva, 1.0 / D)
                        musq = sm.tile([128, na], F32, tag="musqs", name="musqs")
                        nc.vector.tensor_mul(musq[:], mv[:, nd:T, 0], mv[:, nd:T, 0])
                        nc.vector.tensor_sub(mv[:, nd:T, 1], mv[:, nd:T, 1], musq[:])
                    rs = res.tile(
                        [128, T], F32, tag=f"rstd_{name}", name=f"rstd_{name}"
                    )
                    _rsqrt_dve(nc, sm, rs[:], mv[:, :, 1], T, "st")
                    rstd[name] = rs
                    pk = sm.tile(
                        [128, 2 * T], BF16, tag=f"pk_{name}", name=f"pk_{name}"
                    )
                    pkv = pk[:].rearrange("p (t two) -> p t two", two=2)
                    nc.vector.tensor_scalar_mul(pkv[:, :, 0], mv[:, :, 0], -1.0)
                    rcp = sm.tile([128, T], F32, tag="rcp", name="rcp")
                    nc.vector.reciprocal(rcp[:], rs[:])
                    nc.vector.tensor_copy(pkv[:, :, 1], rcp[:])
                    rows = res.tile(
                        [2, T, 128], F8E4, tag=f"augl_{name}", name=f"augl_{name}"
                    )
                    for t in range(T):
                        pst = psum_pool.tile([2, 128], BF16, tag=psum_tag, name="pst")
                        nc.tensor.transpose(
                            pst[:], pk[:, 2 * t : 2 * t + 2], identb[:]
                        )
                        nc.vector.tensor_copy(rows[:, t, :], pst[:])
                    a8 = res.tile(
                        [1, 2, T, 128], F8E4, tag=f"aug8_{name}", name=f"aug8_{name}"
                    )
                    for j in range(2):
                        nc.sync.dma_start(a8[0:1, j, :, :], rows[j : j + 1, :, :])
                    aug8[name] = a8

                do_stats("q", qn_, NLT, pp, "tp", wk, na=3)

                # ---------- weight + aug-row uploads (pre-folded on host) ----
                wpq = wqvw.tile([128, NKT, HE], BF16, tag="wpq", name="wpq")
                nc.sync.dma_start(
                    wpq[:], Wq.ap().rearrange("(k p) n -> p k n", p=128)
                )
                nc.sync.dma_start(
                    wpk[:], Wk.ap().rearrange("(k p) n -> p k n", p=128)
                )
                wpv = res.tile([128, NKT, HE], F8E4, tag="wpv", name="wpv")
                nc.sync.dma_start(
                    wpv[:], Wv.ap().rearrange("(k p) n -> p k n", p=128)
                )
                nc.sync.dma_start(
                    wpo[:], Wo.ap().rearrange("(k p) n -> p k n", p=128)
                )
                for nm, dt_ in (("q", arq_d), ("k", ark_d), ("v", arv_d)):
                    a = res.tile([1, 2, HE], F8E4, tag=f"ar_{nm}", name=f"ar_{nm}")
                    nc.sync.dma_start(a[:], dt_.ap())
                    ar8[nm] = a
                if bias_o:
                    cof = sm.tile([1, D], F32, tag="cof", name="cof")
                    nc.sync.dma_start(cof[:], co_d.ap())
                    nc.vector.tensor_copy(co_b[:], cof[:])
                nc.sync.dma_start(
                    kTt[:], kT.ap().rearrange("(k p) s -> p k s", p=128)
                )
                vTt = res.tile([128, NKT, S], F8E4, tag="vTt", name="vTt")
                nc.sync.dma_start(
                    vTt[:], vT.ap().rearrange("(k p) s -> p k s", p=128)
                )

              # ---------- Q-all projection ----------
              with (
                    tc.tile_pool(name="wq1", bufs=1) as wq1,
                    tc.tile_pool(name="qv", bufs=2, space="PSUM") as qvp,
              ):
                    qTt = wq1.tile([128, NKT, LH], BF16, tag="qTt", name="qTt")
                    nc.scalar.dma_start(
                        qTt[:], qT.ap().rearrange("(k p) s -> p k s", p=128)
                    )
                    for t in range(NLT if "q" not in skip else 0):
                        po = qvp.tile([128, HE], F32, tag="proj", name="poq")
                        for c in range(2):
                            cs = slice(c * 512, (c + 1) * 512)
                            for k in range(NKT):
                                nc.tensor.matmul(
                                    po[:, cs], qTt[:, k, t * 128 : (t + 1) * 128],
                                    wpq[:, k, cs], start=(k == 0), stop=False,
                                )
                            nc.tensor.matmul(
                                po[:, cs], aug8["q"][0:1, :, t, :],
                                ar8["q"][0:1, :, cs],
                                start=False, stop=True, perf_mode=DR,
                            )
                        nc.scalar.activation(
                            out1[:, t, :], po[:], AF.Copy,
                            scale=rstd["q"][:, t : t + 1],
                        )
                        for g in range(2):
                            ptq = qvp.tile([128, 512], BF16, tag="tpq", name="ptq")
                            for j in range(4):
                                i = g * 4 + j
                                nc.tensor.transpose(
                                    ptq[:, j * 128 : (j + 1) * 128],
                                    out1[:, t, i * 128 : (i + 1) * 128],
                                    identb[:],
                                )
                            nc.scalar.activation(
                                QT[:, g * 4 : (g + 1) * 4, t * 128 : (t + 1) * 128],
                                ptq[:].rearrange("p (j f) -> p j f", f=128),
                                AF.Copy,
                            )
                    # shuffle QT into DoubleRow pair-interleave (e = j*32+q)
                    for h in range(2):
                        for jj in range(2):
                            nc.sync.dma_start(
                                QTdr[h * 32 : (h + 1) * 32, :, jj, :],
                                QT[h * 64 + jj * 32 : h * 64 + (jj + 1) * 32, :, :],
                            )
                    # k/v stats stream on DVE/ACT while Q-all runs on PE
                    do_stats("k", kn_, NST, qvp, "tp", wq1, na=5)
                    do_stats("v0", vn_, 8, qvp, "tp", wq1, t0=0, na=3)
                    do_stats("v1", vn_, 8, qvp, "tp", wq1, t0=8, na=3)
                    sck = res.tile([128, NST], F32, tag="sck", name="sck")
                    nc.vector.tensor_scalar_mul(sck[:], rstd["k"][:], SCALE)
                    akt = res.tile([128, NST], F32, tag="akt", name="akt")
                    nc.vector.tensor_scalar_mul(
                        akt[:], rstd["k"][:], SCALE * LOG2E * 8.0
                    )

            # ---------- attention + final stage, per head pair ----------
            with (
                tc.tile_pool(name="kt", bufs=2) as ktp,
                tc.tile_pool(name="ktm", bufs=2) as ktmp,
                tc.tile_pool(name="pt", bufs=4) as ptpool,
                tc.tile_pool(name="mst", bufs=2) as mstp,
                tc.tile_pool(name="sc", bufs=5, space="PSUM") as scp,
                tc.tile_pool(name="av", bufs=2, space="PSUM") as avp,
                tc.tile_pool(name="fin", bufs=1, space="PSUM") as finp,
            ):
                # K proj / final-stage thunks use a dedicated single psum
                # bank so the half-width score tiles keep a 5-deep ring
                def filler_psum():
                    return finp.tile([128, 512], F32, tag="f", name="fill")
                fn_state = {}

                def fn_nonpe(mt):
                    """Spill out1 head-pair columns, gather mixed M, LN stats,
                    normalized o1s. DMA/DVE/ACT only — no PE."""
                    nc.sync.dma_start(
                        out1d.ap().rearrange("(t p) f -> p t f", p=128)[
                            :, :, mt * 128 : (mt + 1) * 128
                        ],
                        out1[:, :, mt * 128 : (mt + 1) * 128],
                    )
                    M = mstp.tile([128, HE], BF16, tag="M", name="M")
                    o1v = out1d.ap().rearrange("(g j) (h e) -> g j h e", j=16, e=64)
                    for ho in range(2):
                        nc.sync.dma_start(
                            M[ho * 64 : (ho + 1) * 64, :].rearrange(
                                "g (j e) -> g j e", e=64
                            ),
                            o1v[:, :, 2 * mt + ho, :],
                        )
                    # ACT-accumulator LN stats of M: Sum(x) and Sum(x^2)
                    # over the free dim, then mu/var/rsqrt on Pool so the
                    # DVE exp stream is untouched at head boundaries.
                    msc = mstp.tile([128, HE], BF16, tag="msc", name="msc")
                    sum0 = sm.tile([128, 1], F32, tag="sum0", name="sum0")
                    sum1 = sm.tile([128, 1], F32, tag="sum1", name="sum1")
                    nc.scalar.activation(msc[:], M[:], AF.Copy, accum_out=sum0[:])
                    nc.scalar.activation(msc[:], M[:], AF.Square, accum_out=sum1[:])
                    mvf = sm.tile([128, 2], F32, tag="mvf", name="mvf")
                    nc.vector.tensor_scalar_mul(mvf[:, 0:1], sum0[:], 1.0 / HE)
                    nc.vector.tensor_scalar_mul(mvf[:, 1:2], sum1[:], 1.0 / HE)
                    musq = sm.tile([128, 1], F32, tag="musq", name="musq")
                    nc.vector.tensor_mul(musq[:], mvf[:, 0:1], mvf[:, 0:1])
                    varf = sm.tile([128, 1], F32, tag="varf", name="varf")
                    nc.vector.tensor_sub(varf[:], mvf[:, 1:2], musq[:])
                    rstdf = sm.tile([128, 1], F32, tag="rstdf", name="rstdf")
                    _rsqrt_dve(nc, sm, rstdf[:], varf[:], 1, "fn")
                    negmu = sm.tile([128, 1], F32, tag="negmu", name="negmu")
                    nc.vector.tensor_scalar_mul(negmu[:], mvf[:, 0:1], -1.0)
                    o1s = mstp.tile([128, HE], BF16, tag="o1s", name="o1s")
                    nc.gpsimd.tensor_scalar(
                        o1s[:], M[:], negmu[:], rstdf[:], op0=ALU.add, op1=ALU.mult
                    )
                    fn_state[mt] = (M, o1s)

                def fn_pe_thunks(mt):
                    """Final-stage PE work for row-tile mt as 4 independent
                    chunks to interleave between attention score groups."""
                    M, o1s = fn_state.pop(mt)
                    o1sT = mstp.tile([128, NKT, 128], BF16, tag="o1sT", name="o1sT")

                    def tp(g):
                        def run():
                            pst = filler_psum()[:, 0:256].bitcast(BF16)
                            for j in range(4):
                                i = g * 4 + j
                                nc.tensor.transpose(
                                    pst[:, j * 128 : (j + 1) * 128],
                                    o1s[:, i * 128 : (i + 1) * 128],
                                    identb[:],
                                )
                            nc.vector.tensor_copy(
                                o1sT[:, g * 4 : (g + 1) * 4, :],
                                pst.rearrange("p (j f) -> p j f", f=128),
                            )
                        return run

                    def db_run(db):
                        def run():
                            ds = slice(db * 512, (db + 1) * 512)
                            fnp = filler_psum()
                            for k in range(NKT):
                                nc.tensor.matmul(
                                    fnp, o1sT[:, k, :], wpo[:, k, ds],
                                    start=(k == 0), stop=False,
                                )
                            if bias_o:
                                nc.tensor.matmul(
                                    fnp, ones_row[:], co_b[:, ds],
                                    start=False, stop=False,
                                )
                            # residual add via identity matmul on PE; copy out
                            # on ACT (PSUM can't be DMA'd directly)
                            nc.tensor.matmul(
                                fnp, identb[:], M[:, ds],
                                start=False, stop=True,
                            )
                            fout = mstp.tile([128, 512], F32, tag="fout", name="fout")
                            nc.scalar.activation(fout[:], fnp, AF.Copy)
                            nc.sync.dma_start(
                                F[mt * 128 : (mt + 1) * 128, ds], fout[:]
                            )
                        return run

                    return [tp(0), tp(1), db_run(0), db_run(1)]

                def k_thunks(hp, KTh):
                    """K projection for head pair hp as 4 chunks; the last
                    chunk shuffles the result into DoubleRow layout."""
                    fsl = slice(hp * 128, (hp + 1) * 128)

                    kdr = kTt[:].rearrange("p (kp kt) s -> p kp kt s", kt=2)
                    wkdr = wpk[:].rearrange("p (kp kt) n -> p kp kt n", kt=2)
                    KTtmp = ktmp.tile([128, S], F8E4, tag="KTt", name="KTtmp")

                    def chain(sq):
                        def run():
                            ss = slice(sq * 512, (sq + 1) * 512)
                            pkps = filler_psum()
                            for kp in range(NKT // 2):
                                nc.tensor.matmul(
                                    pkps, wkdr[:, kp, :, fsl],
                                    kdr[:, kp, :, ss],
                                    start=(kp == 0), stop=False, perf_mode=DR,
                                )
                            nc.tensor.matmul(
                                pkps, ar8["k"][0:1, :, fsl],
                                aug8["k"][0:1, :, sq * 4 : (sq + 1) * 4, :],
                                start=False, stop=True, perf_mode=DR,
                            )
                            if sq % 2 == 0:
                                nc.scalar.activation(KTtmp[:, ss], pkps, AF.Copy)
                            else:
                                nc.vector.tensor_copy(KTtmp[:, ss], pkps)
                            if sq == 3:
                                for h in range(2):
                                    for jj in range(2):
                                        nc.sync.dma_start(
                                            KTh[h * 32 : (h + 1) * 32, jj, :],
                                            KTtmp[
                                                h * 64 + jj * 32 : h * 64
                                                + (jj + 1) * 32,
                                                :,
                                            ],
                                        )
                        return run

                    return [chain(sq) for sq in range(4)]

                def attend(hp, h01, KTh, pe_fillers):
                    h = 2 * hp + h01
                    hs = slice(h01 * 32, (h01 + 1) * 32)
                    pava = avp.tile([128, 4, E + 1], F32, tag="av", name="pava")
                    pavb = avp.tile([128, 4, E + 1], F32, tag="av", name="pavb")
                    pts = {}

                    def av_chain(stp):
                        PT = pts.pop(stp)
                        for lsub in range(NLT):
                            pav = pava if lsub < 4 else pavb
                            nc.tensor.matmul(
                                pav[:, lsub % 4, :],
                                PT[:, :, lsub * 128 : (lsub + 1) * 128],
                                Vaug[:, 2 * stp : 2 * stp + 2, h, :],
                                start=(stp == 0), stop=(stp == NST // 2 - 1),
                                perf_mode=DR,
                            )

                    for stp in range(NST // 2):
                        PT = ptpool.tile([128, 2, LH], F8E4, tag="PT", name="PT")
                        pts[stp] = PT
                        for j in range(2):
                            st = 2 * stp + j
                            eng = EXP_ENG[h01].get(st, "act")
                            for lc in range(2):
                                ls = slice(lc * 512, (lc + 1) * 512)
                                psc = scp.tile(
                                    [128, 512], F32, tag="sc", name="psc"
                                )
                                nc.tensor.matmul(
                                    psc[:],
                                    KTh[hs, :, st * 128 : (st + 1) * 128],
                                    QTdr[hs, hp, :, ls],
                                    start=True, stop=True, perf_mode=DR,
                                )
                                if eng == "act":
                                    nc.scalar.activation(
                                        PT[:, j, ls], psc[:], AF.Exp,
                                        scale=sck[:, st : st + 1],
                                    )
                                else:
                                    nc.vector.tensor_scalar(
                                        PT[:, j, ls].bitcast(I8), psc[:],
                                        akt[:, st : st + 1], 56.0 - CEXP,
                                        op0=ALU.mult, op1=ALU.add,
                                    )
                        if pe_fillers:
                            pe_fillers.pop(0)()
                        if stp >= 1:
                            av_chain(stp - 1)
                    av_chain(NST // 2 - 1)
                    for half, pav in ((0, pava), (1, pavb)):
                        rc4 = sm.tile([128, 4], F32, tag="rc4", name="rc4")
                        nc.vector.reciprocal(rc4[:], pav[:, :, E])
                        for i in range(4):
                            lsub = half * 4 + i
                            nc.vector.scalar_tensor_tensor(
                                out1[:, lsub, h * E : (h + 1) * E],
                                pav[:, i, 0:E],
                                rc4[:, i : i + 1],
                                out1[:, lsub, h * E : (h + 1) * E],
                                op0=ALU.mult, op1=ALU.add,
                            )

                # V projection lives in the attend scope: first half runs
                # inline (scores psum ring), second half fills attend(0,0)
                # slots so exp engines ramp while V finishes on PE.
                vdrr = vTt[:].rearrange("p (kp kt) s -> p kp kt s", kt=2)
                wvdrr = wpv[:].rearrange("p (kp kt) n -> p kp kt n", kt=2)

                def vb_chunk(t):
                    half = "v0" if t < 8 else "v1"
                    tl = t % 8

                    def run():
                        for c in range(2):
                            cs = slice(c * 512, (c + 1) * 512)
                            pv = scp.tile([128, 512], F32, tag="sc", name="pvb")
                            for kp in range(NKT // 2):
                                nc.tensor.matmul(
                                    pv[:],
                                    vdrr[:, kp, :, t * 128 : (t + 1) * 128],
                                    wvdrr[:, kp, :, cs],
                                    start=(kp == 0), stop=False, perf_mode=DR,
                                )
                            nc.tensor.matmul(
                                pv[:], aug8[half][0:1, :, tl, :],
                                ar8["v"][0:1, :, cs],
                                start=False, stop=True, perf_mode=DR,
                            )
                            dst = Vaug[:, t, c * 8 : (c + 1) * 8, 0:E]
                            pvv = pv[:].rearrange("p (h e) -> p h e", e=E)
                            if c == 0:
                                nc.scalar.activation(
                                    dst, pvv, AF.Copy,
                                    scale=rstd[half][:, tl : tl + 1],
                                )
                            else:
                                nc.vector.tensor_scalar_mul(
                                    dst, pvv, rstd[half][:, tl : tl + 1]
                                )
                    return run

                # K(0) projected up front; V second half and K(1) fill the
                # first head pair's score slots; K(hp+1)/FN(hp-1) fill later.
                KThs = {0: ktp.tile([64, 2, S], F8E4, tag="KT", name="KT0")}
                for t in k_thunks(0, KThs[0]):
                    t()
                if "v" not in skip:
                    for t in range(8):
                        vb_chunk(t)()
                    vb = [vb_chunk(t) for t in range(8, 16)]
                else:
                    vb = []
                for hp in range(HP):
                    fill_h0, fill_h1 = [], []
                    if hp == 0:
                        fill_h0 = vb
                        if HP > 1:
                            KThs[1] = ktp.tile(
                                [64, 2, S], F8E4, tag="KT", name="KT"
                            )
                            fill_h1 = k_thunks(1, KThs[1])
                    else:
                        if hp + 1 < HP:
                            KThs[hp + 1] = ktp.tile(
                                [64, 2, S], F8E4, tag="KT", name="KT"
                            )
                            fill_h0 = k_thunks(hp + 1, KThs[hp + 1])
                        if "fin" not in skip:
                            fill_h1 = fn_pe_thunks(hp - 1)
                    if "attn" not in skip:
                        attend(hp, 0, KThs[hp], fill_h0)
                        attend(hp, 1, KThs[hp], fill_h1)
                    else:
                        for t in fill_h0 + fill_h1:
                            t()
                    del KThs[hp]
                    if "fin" not in skip:
                        fn_nonpe(hp)
                if "fin" not in skip:
                    for t in fn_pe_thunks(HP - 1):
                        t()

    nc.compile()
    return nc


_NC_CACHE = {}


def kernel(**inputs):
    import ml_dtypes

    bf16 = ml_dtypes.bfloat16
    f8 = mybir.dt.np(F8E4)
    q = np.asarray(inputs["q"], dtype=np.float32)
    k = np.asarray(inputs["k"], dtype=np.float32)
    v = np.asarray(inputs["v"], dtype=np.float32)

    nw = np.asarray(inputs["norm_w"], np.float32)
    nb = np.asarray(inputs["norm_b"], np.float32)
    n1w = np.asarray(inputs["norm1_w"], np.float32)
    n1b = np.asarray(inputs["norm1_b"], np.float32)
    Wq_f = np.asarray(inputs["Wq"], np.float32)
    Wk_f = np.asarray(inputs["Wk"], np.float32)
    Wv_f = np.asarray(inputs["Wv"], np.float32)
    Wo_f = np.asarray(inputs["Wo"], np.float32)

    # fold LN weight into projection weights; build aug value rows
    wq_s = nw[:, None] * Wq_f
    wk_s = nw[:, None] * Wk_f
    wv_s = nw[:, None] * Wv_f
    wo_s = n1w[:, None] * Wo_f

    def ar_rows(ws, W, bias):
        srow = ws.sum(0)
        crow = nb @ W + np.asarray(bias, np.float32)
        return np.ascontiguousarray(
            np.stack([srow, crow])[None, :, :]
        ).astype(f8)

    co = (n1b @ Wo_f + np.asarray(inputs["bo"], np.float32)).reshape(1, D)
    bias_o = bool(np.any(co))

    key = ("nc", bias_o)
    if key not in _NC_CACHE:
        _NC_CACHE[key] = build_nc(bias_o=bias_o)
        _NC_CACHE["nc"] = _NC_CACHE[key]
    nc = _NC_CACHE[key]

    shared = {
        "Wq": wq_s.astype(bf16),
        "Wk": wk_s.astype(f8),
        "Wv": wv_s.astype(f8),
        "Wo": wo_s.astype(bf16),
        "arq_d": ar_rows(wq_s, Wq_f, inputs["bq"]),
        "ark_d": ar_rows(wk_s, Wk_f, inputs["bk"]),
        "arv_d": ar_rows(wv_s, Wv_f, inputs["bv"]),
        "co_d": np.ascontiguousarray(co),
    }
    in_maps = []
    for c in range(8):
        b, half = c // 2, c % 2
        qb = np.ascontiguousarray(q[b, half * LH : (half + 1) * LH, :])
        in_maps.append(
            dict(
                shared,
                qn_=qb.astype(f8),
                kn_=k[b].astype(f8),
                vn_=v[b].astype(f8),
                qT=np.ascontiguousarray(qb.T).astype(bf16),
                kT=np.ascontiguousarray(k[b].T).astype(f8),
                vT=np.ascontiguousarray(v[b].T).astype(f8),
            )
        )

    res = run_bass_kernel_spmd(nc, in_maps, core_ids=list(range(8)))
    out = np.empty((B, L, D), dtype=np.float32)
    m = np.arange(LH)
    r = (m // 64) * 128 + (m % 64)
    for c in range(8):
        b, half = c // 2, c % 2
        out[b, r + half * 64, :] = res.results[c]["F"]
    return out
